# revision 79
# baseline (speedup 1.0000x reference)
"""Multi-head causal attention (B=2, T=2048, D=1024, H=16) on 8 trn2 NeuronCores.

Sharding: 8 cores = 2 batches x 4 head-groups (4 heads each). Each core:
  - computes qkv projections for its 4 heads from x[b] (pre-transposed on host),
  - runs masked softmax attention,
  - emits a partial output projection y_part = attn_heads @ w_out[head_rows] (bf16).
Host sums the 4 partial y per batch in fp32.

Schedule highlights (tuned against the TimelineSim cost model):
  - PE matmul cost is moving-column count only; attnU runs in (q-partition
    x 65) orientation: stationary = exp-scores sub-tile [128k x 128q],
    moving = [v(64) | ones(1)] so softmax denominators accumulate as psum
    column 64 at full array utilization (no wasted columns).
  - PSUM accumulation groups are per 2KB zero-region (whole bank): one
    start/stop group per head-bank per row; sub-ranges zero-fill on first
    touch.
  - One ACT exp per (pair, qtile, ktile) covering both heads ([128, 2, 512]
    psum tile spanning 2 banks) to halve ACT fixed overhead.
  - Normalization: psum col 64 -> custom-DVE reciprocal -> stride-0
    broadcast multiply into atT (q-part, dims); atT -> at via batched
    128x128-block DMA-transpose for the out-proj stationary. No
    cross-partition shuffles.
  - All (pair, q-row) block streams are flattened into one globally
    software-pipelined sequence (scores lookahead 2, bounded by 2 psS
    bufs = 4 banks) so the exp stream crosses row seams without stalling;
    the final row norms/transposes/out-projects per q-subtile as each
    subtile's accumulation completes at its diagonal block.
  - qkv/out projections and v-tiles are filler units drained into the
    exp-paced attention loop (rationed when scarce so the exp-heavy late
    rows stay covered); rows run in order q0, q1, q3, q2.
  - Input DMAs on one queue in strict need-order (wk/wq pair-0 halves,
    first x quarter split by t-halves, pair-1 weight halves, wv, compact
    pmask window, rest); the prologue runs all-k-then-all-q per half so
    the PE starts on the earliest bytes. pmask carries only the 128-wide
    partial window of each pattern (4x less startup DMA). y stores in
    bf16 (summed in fp32 on host); mask multiplies offloaded to GpSimd.
"""
import sys
sys.path.insert(0, "/opt/trn_rl_repo")

import numpy as np
import ml_dtypes

import concourse.bass as bass
import concourse.mybir as mybir
import concourse.tile as tile
from concourse import bacc
from concourse.bass_utils import run_bass_kernel_spmd

B, T, D, H, Dh = 2, 2048, 1024, 16, 64
P = 128
QT = 512              # q-tile width (score tile free dim)
NQ = T // QT          # 4
NKT = T // P          # 16
ND = D // P           # 8
HPC = 4               # heads per core
NPAIR = HPC // 2      # head pairs per core
NSUB = QT // P        # 4 q-subtiles per q-tile
VW = Dh + 1           # attnU moving width: 64 v dims + 1 ones col
N_CORES = 8

f32 = mybir.dt.float32
bf16 = mybir.dt.bfloat16
CDT = bf16
NP_CDT = ml_dtypes.bfloat16


def _block_structure(mask: np.ndarray):
    """Classify maskT (k,q) blocks: per q-tile a list of (kt, pattern_idx|None).

    For each unique partial pattern derive (w0, m_lo, m_hi): w0 leading
    all-masked columns (exp skipped), [m_lo, m_hi) the column range needing
    the mask multiply.
    """
    maskT = (mask != 0).T.astype(np.float32)  # [k, q] visibility
    vis = []
    patterns = []
    meta = []
    pat_index = {}
    for qt in range(NQ):
        row = []
        for kt in range(NKT):
            blk = maskT[kt * P:(kt + 1) * P, qt * QT:(qt + 1) * QT]
            s = blk.sum()
            if s == 0:
                continue
            if s == blk.size:
                row.append((kt, None))
            else:
                key = blk.tobytes()
                if key not in pat_index:
                    pat_index[key] = len(patterns)
                    patterns.append(blk)
                    col_any = blk.any(axis=0)
                    col_all = blk.all(axis=0)
                    w0 = int(np.argmax(col_any)) if col_any.any() else QT
                    partial_cols = np.nonzero(col_any & ~col_all)[0]
                    if partial_cols.size:
                        m_lo, m_hi = int(partial_cols[0]), int(partial_cols[-1]) + 1
                    else:
                        m_lo = m_hi = 0
                    meta.append((w0, m_lo, m_hi))
                row.append((kt, pat_index[key]))
        vis.append(row)
    if patterns:
        pm = np.stack(patterns)
    else:
        pm = np.zeros((1, P, QT), np.float32)
    return vis, pm, meta


def _row_subs(row, meta):
    """Per q-subtile s: (first_j, last_j) of blocks covering s, or None."""
    sub_first = {}
    sub_last = {}
    for j, (kt, pidx) in enumerate(row):
        w0 = 0 if pidx is None else meta[pidx][0]
        for s in range(w0 // P, NSUB):
            if s not in sub_first:
                sub_first[s] = j
            sub_last[s] = j
    return sub_first, sub_last


def _build_program(vis, n_pm, meta=(), compile=True, debug=False):
    nc = bacc.Bacc() if compile else bass.Bass()
    xT = nc.declare_dram_parameter("xT", [D, T], CDT, isOutput=False)
    wq = nc.declare_dram_parameter("wq", [D, HPC * Dh], CDT, isOutput=False)
    wk = nc.declare_dram_parameter("wk", [D, HPC * Dh], CDT, isOutput=False)
    wv = nc.declare_dram_parameter("wv", [D, HPC * Dh], CDT, isOutput=False)
    wo = nc.declare_dram_parameter("wo", [HPC * Dh, D], CDT, isOutput=False)
    # pmask holds only the P-wide partial window of each pattern (cols
    # [w0, w0+P) of the full tile) — the rest is all-ones/all-zeros
    pmask = nc.declare_dram_parameter("pmask", [n_pm, P, P], CDT, isOutput=False)
    ident = nc.declare_dram_parameter("ident", [P, P], CDT, isOutput=False)
    y = nc.declare_dram_parameter("y", [T, D], CDT, isOutput=True)
    if debug:
        dbg = {
            "d_qT": nc.declare_dram_parameter("d_qT", [NPAIR, P, T], CDT, isOutput=True),
            "d_kT": nc.declare_dram_parameter("d_kT", [NPAIR, P, T], CDT, isOutput=True),
            "d_v1": nc.declare_dram_parameter("d_v1", [P, NKT, HPC, VW], CDT, isOutput=True),
            "d_atT": nc.declare_dram_parameter("d_atT", [NPAIR, P, NKT, P], CDT, isOutput=True),
            "d_at": nc.declare_dram_parameter("d_at", [NPAIR, P, T], CDT, isOutput=True),
        }

    inv_sqrt_dh = 1.0 / float(np.sqrt(Dh))
    ROW_ORDER = (0, 1, 3, 2)

    with tile.TileContext(nc) as tc:
        with (
            tc.tile_pool(name="persist", bufs=1) as persist,
            tc.tile_pool(name="work", bufs=3) as work,
            tc.tile_pool(name="psA", bufs=2, space="PSUM") as psA,
            tc.tile_pool(name="psS", bufs=2, space="PSUM") as psS,
            tc.tile_pool(name="psU", bufs=1, space="PSUM") as psU,
        ):
            # ---- persistent SBUF tensors ----
            xt_sb = persist.tile([P, ND, T], CDT, tag="xt")
            wq_sb = persist.tile([P, ND, HPC * Dh], CDT, tag="wq")
            wk_sb = persist.tile([P, ND, HPC * Dh], CDT, tag="wk")
            wv_sb = persist.tile([P, ND, HPC * Dh], CDT, tag="wv")
            wo_sb = persist.tile([P, NPAIR, D], CDT, tag="wo")
            pm_sb = persist.tile([P, n_pm, P], CDT, tag="pm")
            qT_sb = [persist.tile([P, T], CDT, tag=f"qT{p}", name=f"qT{p}") for p in range(NPAIR)]
            kT_sb = [persist.tile([P, T], CDT, tag=f"kT{p}", name=f"kT{p}") for p in range(NPAIR)]
            # atT: (q-part, per t-tile: 2 heads x 64 dims); at: transposed
            atT_sb = [persist.tile([P, NKT, P], CDT, tag=f"aT{p}", name=f"aT{p}") for p in range(NPAIR)]
            at_sb = [persist.tile([P, T], CDT, tag=f"at{p}", name=f"at{p}") for p in range(NPAIR)]
            # v1: per (k-tile, head): [128 k, 65] = v dims 0:64, ones col 64.
            v1_sb = persist.tile([P, NKT, HPC, VW], CDT, tag="v1")
            id_sb = persist.tile([P, P], CDT, tag="id")

            # ones columns (tiny strided memset; no DMA dependency)
            nc.vector.memset(v1_sb[:, :, :, Dh:VW], 1.0)

            # dummy exp to pre-trigger the ACT exp-table load off the
            # critical path (reads the just-memset ones column)
            scratch = persist.tile([P, 1], CDT, tag="scr")
            nc.scalar.activation(
                scratch[:], v1_sb[:, 0, 0, Dh:VW],
                mybir.ActivationFunctionType.Exp)

            xr = xT.rearrange("(o p) t -> p o t", p=P)
            wqr = wq.rearrange("(o p) e -> p o e", p=P)
            wkr = wk.rearrange("(o p) e -> p o e", p=P)
            # input DMAs on the sync queue, strictly need-ordered. The qk
            # weights are split by head-pair so the prologue (pair 0 only)
            # unblocks after 0.5MB of weights instead of 1MB; pmask is the
            # compact partial window (4x smaller).
            nc.sync.dma_start(wk_sb[:, :, 0:P], wkr[:, :, 0:P])
            nc.sync.dma_start(wq_sb[:, :, 0:P], wqr[:, :, 0:P])
            nc.sync.dma_start(xt_sb[:, :, 0:QT // 2], xr[:, :, 0:QT // 2])
            nc.sync.dma_start(xt_sb[:, :, QT // 2:QT], xr[:, :, QT // 2:QT])
            nc.sync.dma_start(wk_sb[:, :, P:2 * P], wkr[:, :, P:2 * P])
            nc.sync.dma_start(wq_sb[:, :, P:2 * P], wqr[:, :, P:2 * P])
            nc.sync.dma_start(wv_sb[:], wv.rearrange("(o p) e -> p o e", p=P))
            nc.sync.dma_start(pm_sb[:], pmask.rearrange("n p q -> p n q"))
            for c in range(1, NQ):
                nc.sync.dma_start(xt_sb[:, :, c * QT:(c + 1) * QT],
                                  xr[:, :, c * QT:(c + 1) * QT])
            nc.sync.dma_start(wo_sb[:], wo.rearrange("(o p) e -> p o e", p=P))
            nc.sync.dma_start(id_sb[:], ident[:])


            # ---- filler queue: PE-side work interleaved into ACT-paced ----
            # ---- attention steps                                        ----
            fillers = []  # list of (key, thunk); emitted in order

            def drain(k):
                # when fillers run low, ration them: the exp-heavy late rows
                # need leftover PE work to cover their ACT-paced stretches.
                # when the queue is deep, pre-drain harder so seam flushes
                # stay small.
                if len(fillers) > 60:
                    k += 1
                elif k > 1 and len(fillers) < 30:
                    k = 1
                for _ in range(min(k, len(fillers))):
                    fillers.pop(0)[1]()

            def flush_through(pred):
                while any(pred(key) for key, _ in fillers):
                    fillers.pop(0)[1]()

            # ---- v = x @ wv for one t-tile ----
            def emit_v(tt):
                ps_v = psA.tile([P, QT], f32, tag="psA", name=f"psv{tt}")
                for dt in range(ND):
                    nc.tensor.matmul(
                        ps_v[:, :HPC * Dh],
                        xt_sb[:, dt, tt * P:(tt + 1) * P],
                        wv_sb[:, dt, :],
                        start=(dt == 0),
                        stop=(dt == ND - 1),
                    )
                ps_vh = ps_v[:, :HPC * Dh].rearrange("p (h e) -> p h e", h=HPC)
                # alternate DVE/ACT evictions: all-DVE makes the PE's filler
                # stream stall on psA turnover behind the Vector queue
                if tt % 2:
                    nc.scalar.copy(v1_sb[:, tt, :, 0:Dh], ps_vh[:])
                else:
                    nc.vector.tensor_copy(v1_sb[:, tt, :, 0:Dh], ps_vh[:])

            # ---- kT or qT projection for (pair, nt) as 9 filler units ----
            def proj_units(kind, p, nt):
                w_sb = wk_sb if kind == "kT" else wq_sb
                out_sb = kT_sb[p] if kind == "kT" else qT_sb[p]
                ps_box = []

                def mm(dt):
                    if dt == 0:
                        ps_box.append(psA.tile(
                            [P, QT], f32, tag="psA", name=f"ps{kind}{p}_{nt}"))
                    nc.tensor.matmul(
                        ps_box[0],
                        w_sb[:, dt, p * P:(p + 1) * P],
                        xt_sb[:, dt, nt * QT:(nt + 1) * QT],
                        start=(dt == 0),
                        stop=(dt == ND - 1),
                    )

                def evict():
                    if (nt + (0 if kind == "kT" else 1)) % 2:
                        nc.scalar.copy(
                            out_sb[:, nt * QT:(nt + 1) * QT], ps_box[0])
                    else:
                        nc.vector.tensor_copy(
                            out_sb[:, nt * QT:(nt + 1) * QT], ps_box[0])

                key = (kind, p, nt)

                def mk(dt):
                    return lambda: mm(dt)

                units = [(key, mk(dt)) for dt in range(ND)]
                units.append((key, evict))
                return units

            # ---- out-projection for one t-tile/half (as filler) ----
            # both halves share one ysb tile; a single [P, 1024] DMA per
            # t-tile (2KB rows) halves the sync-queue config count
            y_tiles = {}

            def make_outproj(tt, half, pe_t=None):
                def go():
                    if pe_t is not None and half == 0:
                        # last row: transpose this tile's atT on the PE —
                        # drained >= one block after the norm wrote atT, so
                        # the wait is already satisfied and the DMA
                        # round-trip + completion-sem latency is skipped
                        pp, s = pe_t
                        psT = psA.tile([P, P], CDT, tag="psA", name=f"peT{s}")
                        nc.tensor.transpose(
                            psT, atT_sb[pp][:, tt, :], id_sb[:])
                        nc.vector.tensor_copy(
                            at_sb[pp][:, tt * P:(tt + 1) * P], psT)
                    ps_y = psA.tile([P, QT], f32, tag="psA", name=f"psy{tt}_{half}")
                    for p in range(NPAIR):
                        nc.tensor.matmul(
                            ps_y[:],
                            at_sb[p][:, tt * P:(tt + 1) * P],
                            wo_sb[:, p, half * QT:(half + 1) * QT],
                            start=(p == 0),
                            stop=(p == NPAIR - 1),
                        )
                    if pe_t is not None:
                        # last row: store each half as soon as its cast lands
                        # so the final cast and y DMA pipeline
                        yh = work.tile([P, QT], CDT, tag="y", name=f"y{tt}_{half}")
                        nc.vector.tensor_copy(yh[:], ps_y[:])
                        nc.sync.dma_start(
                            y[tt * P:(tt + 1) * P, half * QT:(half + 1) * QT],
                            yh[:])
                        return
                    if tt not in y_tiles:
                        y_tiles[tt] = [
                            work.tile([P, 2, QT], CDT, tag="y", name=f"y{tt}"), 0]
                    ent = y_tiles[tt]
                    nc.vector.tensor_copy(ent[0][:, half], ps_y[:])
                    ent[1] += 1
                    if ent[1] == 2:
                        nc.sync.dma_start(
                            y[tt * P:(tt + 1) * P, :],
                            ent[0].rearrange("p a b -> p (a b)"))
                return go

            # ---- globally pipelined attention: one flat block stream ----
            def norm_emit(rc, lo, hi, pe_t=False):
                p, qt, sub_first, ps_u = rc["p"], rc["qt"], rc["sub_first"], rc["ps_u"]
                n = hi - lo
                den = work.tile([P, 2, n, 1], f32, tag="den", name=f"den{p}_{qt}_{lo}")
                rep = work.tile([P, 2, n, 1], f32, tag="rep", name=f"rep{p}_{qt}_{lo}")
                for h in range(2):
                    nc.vector.tensor_copy(den[:, h], ps_u[h][:, lo:hi, Dh:VW])
                    for s in range(lo, hi):
                        if s not in sub_first:
                            nc.vector.memset(den[:, h, s - lo], 1.0)
                nc.vector.reciprocal_approx_fast(
                    rep.rearrange("p a b c -> p (a b c)"),
                    den.rearrange("p a b c -> p (a b c)"))
                for h in range(2):
                    nc.vector.tensor_mul(
                        atT_sb[p][:, qt * NSUB + lo:qt * NSUB + hi, h * Dh:(h + 1) * Dh],
                        ps_u[h][:, lo:hi, 0:Dh],
                        rep[:, h].broadcast_to((P, n, Dh)),
                    )
                    for s in range(lo, hi):
                        if s not in sub_first:
                            nc.vector.memset(
                                atT_sb[p][:, qt * NSUB + s, h * Dh:(h + 1) * Dh], 0.0)
                if pe_t:
                    # last row: the transpose happens on the PE inside the
                    # out-projection filler — only the normalization here
                    return
                # batched transpose overlaps remaining work
                nc.sync.dma_start_transpose(
                    at_sb[p][:, (qt * NSUB + lo) * P:(qt * NSUB + hi) * P].rearrange(
                        "p (n c) -> p n c", n=n),
                    atT_sb[p][:, qt * NSUB + lo:qt * NSUB + hi, :].rearrange(
                        "p n c -> p (n c)"))

            def sc_emit(rc, j, pieces=None):
                p, qt, row = rc["p"], rc["qt"], rc["row"]
                kt, pidx = row[j]
                if j == 0:
                    flush_through(lambda key, p=p, qt=qt: (
                        key[0] == "qT" and key[1] == p and key[2] == qt))
                flush_through(lambda key, p=p, kt=kt: (
                    key[0] == "kT" and key[1] == p and key[2] <= kt // NSUB))
                w0 = 0 if pidx is None else meta[pidx][0]
                ps_s = psS.tile([P, 2, QT], f32, tag="s", name=f"s_{p}_{qt}_{kt}")
                es = work.tile([P, 2, QT], CDT, tag="es", name=f"es_{p}_{qt}_{kt}")
                if w0 % P:
                    # stale data in the leading partial subtile
                    nc.vector.memset(es[:, :, (w0 // P) * P:w0], 0.0)
                # pieces: column ranges emitted as independent score/exp
                # passes (used by the first blocks so the exp stream starts
                # before the full-width qT projection lands)
                for lo, hi in (pieces or ((w0, QT),)):
                    for h in range(2):
                        base = h * Dh
                        nc.tensor.matmul(
                            ps_s[:, h, lo:hi],
                            kT_sb[p][base:base + Dh, kt * P:(kt + 1) * P],
                            qT_sb[p][base:base + Dh, qt * QT + lo:qt * QT + hi],
                            start=True,
                            stop=True,
                            tile_position=(base, 0),
                        )
                    nc.scalar.activation(
                        es[:, :, lo:hi], ps_s[:, :, lo:hi],
                        mybir.ActivationFunctionType.Exp,
                        scale=inv_sqrt_dh,
                    )
                    if pidx is not None:
                        _w0, m_lo, m_hi = meta[pidx]
                        ml, mh = max(m_lo, lo), min(m_hi, hi)
                        if mh > ml:
                            pmb = pm_sb[:, pidx:pidx + 1,
                                        ml - _w0:mh - _w0].broadcast_to(
                                (P, 2, mh - ml))
                            if rc["last_row"] and j == len(rc["row"]) - 1:
                                # kernel tail: DVE is idle here and faster
                                # per element than GpSimd — shortens the
                                # final block's exp->mask->attnU chain
                                nc.vector.tensor_mul(
                                    es[:, :, ml:mh], es[:, :, ml:mh], pmb)
                            else:
                                nc.gpsimd.tensor_mul(
                                    es[:, :, ml:mh], es[:, :, ml:mh], pmb)
                rc["es_q"][j] = (es, w0)

            # PSUM accumulation groups are per 2KB zero-region (a whole
            # bank): start marks the bank pending-zero, each later matmul
            # zero-fills its range on first touch and accumulates after.
            # So: one group per head-bank per row.
            def au_emit(rc, j):
                p, qt, row = rc["p"], rc["qt"], rc["row"]
                kt, pidx = row[j]
                flush_through(lambda key, kt=kt: (
                    key[0] == "v" and key[1] <= kt))
                if rc["ps_u"] is None:
                    rc["ps_u"] = [
                        psU.tile([P, NSUB, VW], f32, tag=f"u{h}", name=f"u{h}_{p}_{qt}")
                        for h in range(2)
                    ]
                es, w0 = rc["es_q"].pop(j)
                s0 = w0 // P
                last_j = len(row) - 1
                # emit the mask-dependent subtile of both heads LAST: the
                # mask-free subtiles then stream on the in-order PE queue
                # while the GpSimd mask multiply finishes, instead of
                # queueing blocked behind it
                subs = list(range(s0, NSUB))
                if (pidx is not None and meta[pidx][2] > meta[pidx][1]
                        and len(subs) > 1):
                    seq = [(h, s) for s in subs[1:] for h in range(2)]
                    seq += [(h, subs[0]) for h in range(2)]
                else:
                    seq = [(h, s) for h in range(2) for s in subs]
                firsts = {}
                lasts = {}
                for h, s in seq:
                    firsts.setdefault(h, (h, s))
                    lasts[h] = (h, s)
                for h, s in seq:
                    nc.tensor.matmul(
                        rc["ps_u"][h][:, s, :],
                        es[:, h, s * P:(s + 1) * P],
                        v1_sb[:, kt, 2 * p + h, :],
                        start=(j == 0 and (h, s) == firsts[h]),
                        stop=(j == last_j and (h, s) == lasts[h]),
                    )
                if rc["last_row"]:
                    # per-sub pipeline: as each q-subtile completes (its
                    # diagonal block), norm + PE-transpose + out-projection
                    # run while the remaining exps stream
                    while (rc["normed"] < NSUB
                           and rc["sub_last"].get(rc["normed"]) == j):
                        s = rc["normed"]
                        norm_emit(rc, s, s + 1, pe_t=True)
                        tt = qt * NSUB + s
                        fillers.extend(
                            (("op", tt, half),
                             make_outproj(tt, half, pe_t=(p, s)))
                            for half in range(2))
                        rc["normed"] += 1
                if j == last_j:
                    if rc["normed"] < NSUB:
                        norm_emit(rc, rc["normed"], NSUB, pe_t=rc["last_row"])
                        if rc["last_row"]:
                            for s in range(rc["normed"], NSUB):
                                tt = qt * NSUB + s
                                fillers.extend(
                                    (("op", tt, half),
                                     make_outproj(tt, half, pe_t=(p, s)))
                                    for half in range(2))
                        rc["normed"] = NSUB
                    done_pairs[qt] += 1
                    if done_pairs[qt] == NPAIR and qt != ROW_ORDER[-1]:
                        fillers.extend(
                            (("op", tt, half), make_outproj(tt, half))
                            for tt in range(qt * NSUB, (qt + 1) * NSUB)
                            for half in range(2))

            # ---- prologue: inline just enough for row (p0, q0) ----
            # kT/qT in two t-halves (one accumulation group each, ranges
            # zero-fill on first touch): the first half only needs the first
            # x t-piece, so q0's first score/exp pieces start ~5us earlier
            pro_k = psA.tile([P, QT], f32, tag="psA", name="pro_k")
            pro_q = psA.tile([P, QT], f32, tag="psA", name="pro_q")
            HQ = QT // 2
            for hh in range(2):
                # all-k then all-q per half: k needs only wk (first DMA) +
                # the x piece, q waits on the later wq load — so the PE
                # starts on the earliest bytes
                for w_sb, box in ((wk_sb, pro_k), (wq_sb, pro_q)):
                    for dt in range(ND):
                        nc.tensor.matmul(
                            box[:, hh * HQ:(hh + 1) * HQ],
                            w_sb[:, dt, 0:P],
                            xt_sb[:, dt, hh * HQ:(hh + 1) * HQ],
                            start=(hh == 0 and dt == 0),
                            stop=(hh == 1 and dt == ND - 1),
                        )
                # split evicts across engines: ACT is idle before the exps
                nc.vector.tensor_copy(
                    kT_sb[0][:, hh * HQ:(hh + 1) * HQ], pro_k[:, hh * HQ:(hh + 1) * HQ])
                nc.scalar.copy(
                    qT_sb[0][:, hh * HQ:(hh + 1) * HQ], pro_q[:, hh * HQ:(hh + 1) * HQ])

            def v_units(lo, hi):
                return [(("v", tt), (lambda tt=tt: emit_v(tt))) for tt in range(lo, hi)]

            # filler queue ordered to match row order q0, q1, q3, q2 so lazy
            # flushes stay small. pair-1's q0 projections lead: they are
            # data-ready with the prologue (wq/wk/xA), while v0..3 wait the
            # later wv DMA — this kills the first-seam exp bubble.
            fillers.extend(proj_units("kT", 1, 0))
            fillers.extend(proj_units("qT", 1, 0))
            fillers.extend(v_units(0, 4))
            fillers.extend(v_units(4, 8))
            for pp in range(NPAIR):
                fillers.extend(proj_units("kT", pp, 1))
                fillers.extend(proj_units("qT", pp, 1))
            fillers.extend(v_units(8, 12))
            fillers.extend(proj_units("kT", 0, 2))
            fillers.extend(proj_units("kT", 0, 3))
            fillers.extend(proj_units("qT", 0, 3))
            fillers.extend(v_units(12, 16))
            fillers.extend(proj_units("kT", 1, 2))
            fillers.extend(proj_units("kT", 1, 3))
            fillers.extend(proj_units("qT", 1, 3))
            fillers.extend(proj_units("qT", 0, 2))
            fillers.extend(proj_units("qT", 1, 2))

            # ---- flat block stream across all rows (q0, q1, q3, q2) ----
            rows = [(p, qt) for qt in ROW_ORDER for p in range(NPAIR)]
            done_pairs = {qt: 0 for qt in ROW_ORDER}
            flat = []
            for p, qt in rows:
                row = vis[qt]
                last_row = (qt == ROW_ORDER[-1] and p == NPAIR - 1)
                if not row:
                    for h in range(2):
                        nc.vector.memset(
                            atT_sb[p][:, qt * NSUB:(qt + 1) * NSUB,
                                      h * Dh:(h + 1) * Dh], 0.0)
                    nc.sync.dma_start_transpose(
                        at_sb[p][:, qt * NSUB * P:(qt + 1) * NSUB * P].rearrange(
                            "p (n c) -> p n c", n=NSUB),
                        atT_sb[p][:, qt * NSUB:(qt + 1) * NSUB, :].rearrange(
                            "p n c -> p (n c)"))
                    done_pairs[qt] += 1
                    if done_pairs[qt] == NPAIR and qt != ROW_ORDER[-1]:
                        fillers.extend(
                            (("op", tt, half), make_outproj(tt, half))
                            for tt in range(qt * NSUB, (qt + 1) * NSUB)
                            for half in range(2))
                    continue
                sub_first, sub_last = _row_subs(row, meta)
                rc = dict(p=p, qt=qt, row=row, sub_first=sub_first,
                          sub_last=sub_last, ps_u=None, es_q={}, normed=0,
                          last_row=last_row)
                flat.extend((rc, j) for j in range(len(row)))

            def _first_pieces(rc, j):
                _kt, pidx = rc["row"][j]
                w0 = 0 if pidx is None else meta[pidx][0]
                return ((w0, HQ), (HQ, QT)) if w0 < HQ else None

            # the first two blocks emit scores/exps in column halves so the
            # exp stream starts as soon as the first qT half is projected
            if flat:
                sc_emit(*flat[0], pieces=_first_pieces(*flat[0]))
            if len(flat) > 1:
                sc_emit(*flat[1], pieces=_first_pieces(*flat[1]))
            for k, (rc, j) in enumerate(flat):
                # scores first so the exp stream never queues behind chunky
                # filler matmuls on the PE
                if k + 2 < len(flat):
                    sc_emit(*flat[k + 2])
                drain(2)
                au_emit(rc, j)
            while fillers:
                drain(len(fillers))
            if debug:
                for p in range(NPAIR):
                    nc.sync.dma_start(dbg["d_qT"][p], qT_sb[p][:])
                    nc.sync.dma_start(dbg["d_kT"][p], kT_sb[p][:])
                    nc.sync.dma_start(dbg["d_atT"][p], atT_sb[p][:])
                    nc.sync.dma_start(dbg["d_at"][p], at_sb[p][:])
                nc.sync.dma_start(dbg["d_v1"][:], v1_sb[:])
    if compile:
        nc.compile()
    return nc


def _host_inputs(x, mask, w_qkv, w_out):
    vis, pm, meta = _block_structure(np.asarray(mask))
    if meta:
        for w0, m_lo, m_hi in meta:
            assert w0 <= m_lo and m_hi <= w0 + P
        pm_win = np.stack([pm[k][:, meta[k][0]:meta[k][0] + P]
                           for k in range(pm.shape[0])])
    else:
        pm_win = pm[:, :, :P]
    pm_c = pm_win.astype(NP_CDT)
    wq_f, wk_f, wv_f = np.split(np.asarray(w_qkv, np.float32), 3, axis=1)
    in_maps = []
    for core in range(N_CORES):
        b = core // 4
        g = core % 4
        cols = slice(g * HPC * Dh, (g + 1) * HPC * Dh)
        in_maps.append({
            "xT": np.ascontiguousarray(np.asarray(x[b], np.float32).T).astype(NP_CDT),
            "wq": wq_f[:, cols].astype(NP_CDT),
            "wk": wk_f[:, cols].astype(NP_CDT),
            "wv": wv_f[:, cols].astype(NP_CDT),
            "wo": np.asarray(w_out, np.float32)[cols, :].astype(NP_CDT),
            "pmask": pm_c,
            "ident": np.eye(P, dtype=NP_CDT),
        })
    return vis, pm, meta, in_maps


def run(x, mask, w_qkv, w_out, trace=False):
    import os
    vis, pm, meta, in_maps = _host_inputs(x, mask, w_qkv, w_out)
    nc = _build_program(vis, pm.shape[0], meta)
    if not trace:
        os.environ["BASS_NEVER_TRACE"] = "1"
    else:
        os.environ.pop("BASS_NEVER_TRACE", None)
    res = run_bass_kernel_spmd(nc, in_maps, core_ids=list(range(N_CORES)), trace=trace)
    parts = [res.results[i]["y"].astype(np.float32) for i in range(N_CORES)]
    out = np.stack([
        parts[0] + parts[1] + parts[2] + parts[3],
        parts[4] + parts[5] + parts[6] + parts[7],
    ]).astype(np.float32)
    return out, res


def kernel(x, mask, w_qkv, w_out):
    out, _ = run(x, mask, w_qkv, w_out, trace=False)
    return out



# revision 80
# speedup vs baseline: 1.0224x; 1.0224x over previous
"""Multi-head causal attention (B=2, T=2048, D=1024, H=16) on 8 trn2 NeuronCores.

Sharding: 8 cores = 2 batches x 4 head-groups (4 heads each). Each core:
  - computes qkv projections for its 4 heads from x[b] (pre-transposed on host),
  - runs masked softmax attention,
  - emits a partial output projection y_part = attn_heads @ w_out[head_rows] (bf16).
Host sums the 4 partial y per batch in fp32.

Schedule highlights (tuned against the TimelineSim cost model):
  - PE matmul cost is moving-column count only; attnU runs in (q-partition
    x 65) orientation: stationary = exp-scores sub-tile [128k x 128q],
    moving = [v(64) | ones(1)] so softmax denominators accumulate as psum
    column 64 at full array utilization (no wasted columns).
  - PSUM accumulation groups are per 2KB zero-region (whole bank): one
    start/stop group per head-bank per row; sub-ranges zero-fill on first
    touch.
  - One ACT exp per (pair, qtile, ktile) covering both heads ([128, 2, 512]
    psum tile spanning 2 banks) to halve ACT fixed overhead.
  - Normalization: psum col 64 -> custom-DVE reciprocal -> stride-0
    broadcast multiply into atT (q-part, dims); atT -> at via batched
    128x128-block DMA-transpose for the out-proj stationary. No
    cross-partition shuffles.
  - All (pair, q-row) block streams are flattened into one globally
    software-pipelined sequence (scores lookahead 2, bounded by 2 psS
    bufs = 4 banks) so the exp stream crosses row seams without stalling;
    the final row norms/transposes/out-projects per q-subtile as each
    subtile's accumulation completes at its diagonal block.
  - qkv/out projections and v-tiles are filler units drained into the
    exp-paced attention loop (rationed when scarce so the exp-heavy late
    rows stay covered); rows run in order q0, q1, q3, q2.
  - Input DMAs on one queue in strict need-order (wk/wq pair-0 halves,
    first x quarter split by t-halves, pair-1 weight halves, wv, compact
    pmask window, rest); the prologue runs all-k-then-all-q per half so
    the PE starts on the earliest bytes. pmask carries only the 128-wide
    partial window of each pattern (4x less startup DMA). y stores in
    bf16 (summed in fp32 on host); mask multiplies offloaded to GpSimd.
"""
import sys
sys.path.insert(0, "/opt/trn_rl_repo")

import numpy as np
import ml_dtypes

import concourse.bass as bass
import concourse.mybir as mybir
import concourse.tile as tile
from concourse import bacc
from concourse.bass_utils import run_bass_kernel_spmd

B, T, D, H, Dh = 2, 2048, 1024, 16, 64
P = 128
QT = 512              # q-tile width (score tile free dim)
NQ = T // QT          # 4
NKT = T // P          # 16
ND = D // P           # 8
HPC = 4               # heads per core
NPAIR = HPC // 2      # head pairs per core
NSUB = QT // P        # 4 q-subtiles per q-tile
VW = Dh + 1           # attnU moving width: 64 v dims + 1 ones col
N_CORES = 8

f32 = mybir.dt.float32
bf16 = mybir.dt.bfloat16
CDT = bf16
NP_CDT = ml_dtypes.bfloat16


def _block_structure(mask: np.ndarray):
    """Classify maskT (k,q) blocks: per q-tile a list of (kt, pattern_idx|None).

    For each unique partial pattern derive (w0, m_lo, m_hi): w0 leading
    all-masked columns (exp skipped), [m_lo, m_hi) the column range needing
    the mask multiply.
    """
    maskT = (mask != 0).T.astype(np.float32)  # [k, q] visibility
    vis = []
    patterns = []
    meta = []
    pat_index = {}
    for qt in range(NQ):
        row = []
        for kt in range(NKT):
            blk = maskT[kt * P:(kt + 1) * P, qt * QT:(qt + 1) * QT]
            s = blk.sum()
            if s == 0:
                continue
            if s == blk.size:
                row.append((kt, None))
            else:
                key = blk.tobytes()
                if key not in pat_index:
                    pat_index[key] = len(patterns)
                    patterns.append(blk)
                    col_any = blk.any(axis=0)
                    col_all = blk.all(axis=0)
                    w0 = int(np.argmax(col_any)) if col_any.any() else QT
                    partial_cols = np.nonzero(col_any & ~col_all)[0]
                    if partial_cols.size:
                        m_lo, m_hi = int(partial_cols[0]), int(partial_cols[-1]) + 1
                    else:
                        m_lo = m_hi = 0
                    meta.append((w0, m_lo, m_hi))
                row.append((kt, pat_index[key]))
        vis.append(row)
    if patterns:
        pm = np.stack(patterns)
    else:
        pm = np.zeros((1, P, QT), np.float32)
    return vis, pm, meta


def _row_subs(row, meta):
    """Per q-subtile s: (first_j, last_j) of blocks covering s, or None."""
    sub_first = {}
    sub_last = {}
    for j, (kt, pidx) in enumerate(row):
        w0 = 0 if pidx is None else meta[pidx][0]
        for s in range(w0 // P, NSUB):
            if s not in sub_first:
                sub_first[s] = j
            sub_last[s] = j
    return sub_first, sub_last


def _build_program(vis, n_pm, meta=(), compile=True, debug=False):
    nc = bacc.Bacc() if compile else bass.Bass()
    xT = nc.declare_dram_parameter("xT", [D, T], CDT, isOutput=False)
    wq = nc.declare_dram_parameter("wq", [D, HPC * Dh], CDT, isOutput=False)
    wk = nc.declare_dram_parameter("wk", [D, HPC * Dh], CDT, isOutput=False)
    wv = nc.declare_dram_parameter("wv", [D, HPC * Dh], CDT, isOutput=False)
    wo = nc.declare_dram_parameter("wo", [HPC * Dh, D], CDT, isOutput=False)
    # pmask holds only the P-wide partial window of each pattern (cols
    # [w0, w0+P) of the full tile) — the rest is all-ones/all-zeros
    pmask = nc.declare_dram_parameter("pmask", [n_pm, P, P], CDT, isOutput=False)
    ident = nc.declare_dram_parameter("ident", [P, P], CDT, isOutput=False)
    y = nc.declare_dram_parameter("y", [T, D], CDT, isOutput=True)
    if debug:
        dbg = {
            "d_qT": nc.declare_dram_parameter("d_qT", [NPAIR, P, T], CDT, isOutput=True),
            "d_kT": nc.declare_dram_parameter("d_kT", [NPAIR, P, T], CDT, isOutput=True),
            "d_v1": nc.declare_dram_parameter("d_v1", [P, NKT, HPC, VW], CDT, isOutput=True),
            "d_atT": nc.declare_dram_parameter("d_atT", [NPAIR, P, NKT, P], CDT, isOutput=True),
            "d_at": nc.declare_dram_parameter("d_at", [NPAIR, P, T], CDT, isOutput=True),
        }

    inv_sqrt_dh = 1.0 / float(np.sqrt(Dh))
    ROW_ORDER = (0, 1, 3, 2)

    with tile.TileContext(nc) as tc:
        with (
            tc.tile_pool(name="persist", bufs=1) as persist,
            tc.tile_pool(name="work", bufs=3) as work,
            tc.tile_pool(name="psA", bufs=2, space="PSUM") as psA,
            tc.tile_pool(name="psS", bufs=2, space="PSUM") as psS,
            tc.tile_pool(name="psU", bufs=1, space="PSUM") as psU,
        ):
            # ---- persistent SBUF tensors ----
            xt_sb = persist.tile([P, ND, T], CDT, tag="xt")
            wq_sb = persist.tile([P, ND, HPC * Dh], CDT, tag="wq")
            wk_sb = persist.tile([P, ND, HPC * Dh], CDT, tag="wk")
            wv_sb = persist.tile([P, ND, HPC * Dh], CDT, tag="wv")
            wo_sb = persist.tile([P, NPAIR, D], CDT, tag="wo")
            pm_sb = persist.tile([P, n_pm, P], CDT, tag="pm")
            qT_sb = [persist.tile([P, T], CDT, tag=f"qT{p}", name=f"qT{p}") for p in range(NPAIR)]
            kT_sb = [persist.tile([P, T], CDT, tag=f"kT{p}", name=f"kT{p}") for p in range(NPAIR)]
            # atT: (q-part, per t-tile: 2 heads x 64 dims); at: transposed
            atT_sb = [persist.tile([P, NKT, P], CDT, tag=f"aT{p}", name=f"aT{p}") for p in range(NPAIR)]
            at_sb = [persist.tile([P, T], CDT, tag=f"at{p}", name=f"at{p}") for p in range(NPAIR)]
            # v1: per (k-tile, head): [128 k, 65] = v dims 0:64, ones col 64.
            v1_sb = persist.tile([P, NKT, HPC, VW], CDT, tag="v1")
            id_sb = persist.tile([P, P], CDT, tag="id")

            # ones columns (tiny strided memset; no DMA dependency)
            nc.vector.memset(v1_sb[:, :, :, Dh:VW], 1.0)

            # dummy exp to pre-trigger the ACT exp-table load off the
            # critical path (reads the just-memset ones column)
            scratch = persist.tile([P, 1], CDT, tag="scr")
            nc.scalar.activation(
                scratch[:], v1_sb[:, 0, 0, Dh:VW],
                mybir.ActivationFunctionType.Exp)

            xr = xT.rearrange("(o p) t -> p o t", p=P)
            wqr = wq.rearrange("(o p) e -> p o e", p=P)
            wkr = wk.rearrange("(o p) e -> p o e", p=P)
            # input DMAs on the sync queue, strictly need-ordered. The qk
            # weights are split by head-pair so the prologue (pair 0 only)
            # unblocks after 0.5MB of weights instead of 1MB; pmask is the
            # compact partial window (4x smaller).
            nc.sync.dma_start(wk_sb[:, :, 0:P], wkr[:, :, 0:P])
            nc.sync.dma_start(wq_sb[:, :, 0:P], wqr[:, :, 0:P])
            nc.sync.dma_start(xt_sb[:, :, 0:QT // 2], xr[:, :, 0:QT // 2])
            nc.sync.dma_start(xt_sb[:, :, QT // 2:QT], xr[:, :, QT // 2:QT])
            nc.sync.dma_start(wk_sb[:, :, P:2 * P], wkr[:, :, P:2 * P])
            nc.sync.dma_start(wq_sb[:, :, P:2 * P], wqr[:, :, P:2 * P])
            nc.sync.dma_start(wv_sb[:], wv.rearrange("(o p) e -> p o e", p=P))
            nc.sync.dma_start(pm_sb[:], pmask.rearrange("n p q -> p n q"))
            for c in range(1, NQ):
                nc.sync.dma_start(xt_sb[:, :, c * QT:(c + 1) * QT],
                                  xr[:, :, c * QT:(c + 1) * QT])
            nc.sync.dma_start(wo_sb[:], wo.rearrange("(o p) e -> p o e", p=P))
            nc.sync.dma_start(id_sb[:], ident[:])


            # ---- filler queue: PE-side work interleaved into ACT-paced ----
            # ---- attention steps                                        ----
            fillers = []  # list of (key, thunk); emitted in order

            def drain(k):
                # when fillers run low, ration them: the exp-heavy late rows
                # need leftover PE work to cover their ACT-paced stretches.
                # when the queue is deep, pre-drain harder so seam flushes
                # stay small.
                if len(fillers) > 60:
                    k += 1
                elif k > 1 and len(fillers) < 30:
                    k = 1
                for _ in range(min(k, len(fillers))):
                    fillers.pop(0)[1]()

            def flush_through(pred):
                while any(pred(key) for key, _ in fillers):
                    fillers.pop(0)[1]()

            # ---- v = x @ wv for one t-tile ----
            def emit_v(tt):
                ps_v = psA.tile([P, QT], f32, tag="psA", name=f"psv{tt}")
                for dt in range(ND):
                    nc.tensor.matmul(
                        ps_v[:, :HPC * Dh],
                        xt_sb[:, dt, tt * P:(tt + 1) * P],
                        wv_sb[:, dt, :],
                        start=(dt == 0),
                        stop=(dt == ND - 1),
                    )
                ps_vh = ps_v[:, :HPC * Dh].rearrange("p (h e) -> p h e", h=HPC)
                # alternate DVE/ACT evictions: all-DVE makes the PE's filler
                # stream stall on psA turnover behind the Vector queue
                if tt % 2:
                    nc.scalar.copy(v1_sb[:, tt, :, 0:Dh], ps_vh[:])
                else:
                    nc.vector.tensor_copy(v1_sb[:, tt, :, 0:Dh], ps_vh[:])

            # ---- kT or qT projection for (pair, nt) as 9 filler units ----
            def proj_units(kind, p, nt):
                w_sb = wk_sb if kind == "kT" else wq_sb
                out_sb = kT_sb[p] if kind == "kT" else qT_sb[p]
                ps_box = []

                def mm(dt):
                    if dt == 0:
                        ps_box.append(psA.tile(
                            [P, QT], f32, tag="psA", name=f"ps{kind}{p}_{nt}"))
                    nc.tensor.matmul(
                        ps_box[0],
                        w_sb[:, dt, p * P:(p + 1) * P],
                        xt_sb[:, dt, nt * QT:(nt + 1) * QT],
                        start=(dt == 0),
                        stop=(dt == ND - 1),
                    )

                def evict():
                    if (nt + (0 if kind == "kT" else 1)) % 2:
                        nc.scalar.copy(
                            out_sb[:, nt * QT:(nt + 1) * QT], ps_box[0])
                    else:
                        nc.vector.tensor_copy(
                            out_sb[:, nt * QT:(nt + 1) * QT], ps_box[0])

                key = (kind, p, nt)

                def mk(dt):
                    return lambda: mm(dt)

                units = [(key, mk(dt)) for dt in range(ND)]
                units.append((key, evict))
                return units

            # ---- out-projection for one t-tile/half (as filler) ----
            # both halves share one ysb tile; a single [P, 1024] DMA per
            # t-tile (2KB rows) halves the sync-queue config count
            y_tiles = {}

            def make_outproj(tt, half, pe_t=None):
                def go():
                    if pe_t is not None and half == 0:
                        # last row: transpose this tile's atT on the PE —
                        # drained >= one block after the norm wrote atT, so
                        # the wait is already satisfied and the DMA
                        # round-trip + completion-sem latency is skipped
                        pp, s = pe_t
                        psT = psA.tile([P, P], CDT, tag="psA", name=f"peT{s}")
                        nc.tensor.transpose(
                            psT, atT_sb[pp][:, tt, :], id_sb[:])
                        nc.vector.tensor_copy(
                            at_sb[pp][:, tt * P:(tt + 1) * P], psT)
                    ps_y = psA.tile([P, QT], f32, tag="psA", name=f"psy{tt}_{half}")
                    for p in range(NPAIR):
                        nc.tensor.matmul(
                            ps_y[:],
                            at_sb[p][:, tt * P:(tt + 1) * P],
                            wo_sb[:, p, half * QT:(half + 1) * QT],
                            start=(p == 0),
                            stop=(p == NPAIR - 1),
                        )
                    if pe_t is not None:
                        # last row: store each half as soon as its cast lands
                        # so the final cast and y DMA pipeline
                        yh = work.tile([P, QT], CDT, tag="y", name=f"y{tt}_{half}")
                        nc.vector.tensor_copy(yh[:], ps_y[:])
                        nc.sync.dma_start(
                            y[tt * P:(tt + 1) * P, half * QT:(half + 1) * QT],
                            yh[:])
                        return
                    if tt not in y_tiles:
                        y_tiles[tt] = [
                            work.tile([P, 2, QT], CDT, tag="y", name=f"y{tt}"), 0]
                    ent = y_tiles[tt]
                    nc.vector.tensor_copy(ent[0][:, half], ps_y[:])
                    ent[1] += 1
                    if ent[1] == 2:
                        nc.sync.dma_start(
                            y[tt * P:(tt + 1) * P, :],
                            ent[0].rearrange("p a b -> p (a b)"))
                return go

            # ---- globally pipelined attention: one flat block stream ----
            def norm_emit(rc, lo, hi, pe_t=False):
                p, qt, sub_first, ps_u = rc["p"], rc["qt"], rc["sub_first"], rc["ps_u"]
                n = hi - lo
                den = work.tile([P, 2, n, 1], f32, tag="den", name=f"den{p}_{qt}_{lo}")
                rep = work.tile([P, 2, n, 1], f32, tag="rep", name=f"rep{p}_{qt}_{lo}")
                for h in range(2):
                    nc.vector.tensor_copy(den[:, h], ps_u[h][:, lo:hi, Dh:VW])
                    for s in range(lo, hi):
                        if s not in sub_first:
                            nc.vector.memset(den[:, h, s - lo], 1.0)
                nc.vector.reciprocal_approx_fast(
                    rep.rearrange("p a b c -> p (a b c)"),
                    den.rearrange("p a b c -> p (a b c)"))
                for h in range(2):
                    nc.vector.tensor_mul(
                        atT_sb[p][:, qt * NSUB + lo:qt * NSUB + hi, h * Dh:(h + 1) * Dh],
                        ps_u[h][:, lo:hi, 0:Dh],
                        rep[:, h].broadcast_to((P, n, Dh)),
                    )
                    for s in range(lo, hi):
                        if s not in sub_first:
                            nc.vector.memset(
                                atT_sb[p][:, qt * NSUB + s, h * Dh:(h + 1) * Dh], 0.0)
                if pe_t:
                    # last row: the transpose happens on the PE inside the
                    # out-projection filler — only the normalization here
                    return
                # batched transpose overlaps remaining work
                nc.sync.dma_start_transpose(
                    at_sb[p][:, (qt * NSUB + lo) * P:(qt * NSUB + hi) * P].rearrange(
                        "p (n c) -> p n c", n=n),
                    atT_sb[p][:, qt * NSUB + lo:qt * NSUB + hi, :].rearrange(
                        "p n c -> p (n c)"))

            def sc_emit(rc, j, pieces=None):
                p, qt, row = rc["p"], rc["qt"], rc["row"]
                kt, pidx = row[j]
                if j == 0:
                    flush_through(lambda key, p=p, qt=qt: (
                        key[0] == "qT" and key[1] == p and key[2] == qt))
                flush_through(lambda key, p=p, kt=kt: (
                    key[0] == "kT" and key[1] == p and key[2] <= kt // NSUB))
                w0 = 0 if pidx is None else meta[pidx][0]
                ps_s = psS.tile([P, 2, QT], f32, tag="s", name=f"s_{p}_{qt}_{kt}")
                es = work.tile([P, 2, QT], CDT, tag="es", name=f"es_{p}_{qt}_{kt}")
                if w0 % P:
                    # stale data in the leading partial subtile
                    nc.vector.memset(es[:, :, (w0 // P) * P:w0], 0.0)
                # pieces: column ranges emitted as independent score/exp
                # passes (used by the first blocks so the exp stream starts
                # before the full-width qT projection lands)
                for lo, hi in (pieces or ((w0, QT),)):
                    for h in range(2):
                        base = h * Dh
                        nc.tensor.matmul(
                            ps_s[:, h, lo:hi],
                            kT_sb[p][base:base + Dh, kt * P:(kt + 1) * P],
                            qT_sb[p][base:base + Dh, qt * QT + lo:qt * QT + hi],
                            start=True,
                            stop=True,
                            tile_position=(base, 0),
                        )
                    nc.scalar.activation(
                        es[:, :, lo:hi], ps_s[:, :, lo:hi],
                        mybir.ActivationFunctionType.Exp,
                        scale=inv_sqrt_dh,
                    )
                    if pidx is not None:
                        _w0, m_lo, m_hi = meta[pidx]
                        ml, mh = max(m_lo, lo), min(m_hi, hi)
                        if mh > ml:
                            pmb = pm_sb[:, pidx:pidx + 1,
                                        ml - _w0:mh - _w0].broadcast_to(
                                (P, 2, mh - ml))
                            if rc["last_row"] and j == len(rc["row"]) - 1:
                                # kernel tail: DVE is idle here and faster
                                # per element than GpSimd — shortens the
                                # final block's exp->mask->attnU chain
                                nc.vector.tensor_mul(
                                    es[:, :, ml:mh], es[:, :, ml:mh], pmb)
                            else:
                                nc.gpsimd.tensor_mul(
                                    es[:, :, ml:mh], es[:, :, ml:mh], pmb)
                rc["es_q"][j] = (es, w0)

            # PSUM accumulation groups are per 2KB zero-region (a whole
            # bank): start marks the bank pending-zero, each later matmul
            # zero-fills its range on first touch and accumulates after.
            # So: one group per head-bank per row.
            def au_emit(rc, j):
                p, qt, row = rc["p"], rc["qt"], rc["row"]
                kt, pidx = row[j]
                flush_through(lambda key, kt=kt: (
                    key[0] == "v" and key[1] <= kt))
                if rc["ps_u"] is None:
                    rc["ps_u"] = [
                        psU.tile([P, NSUB, VW], f32, tag=f"u{h}", name=f"u{h}_{p}_{qt}")
                        for h in range(2)
                    ]
                es, w0 = rc["es_q"].pop(j)
                s0 = w0 // P
                last_j = len(row) - 1
                for h in range(2):
                    for s in range(s0, NSUB):
                        nc.tensor.matmul(
                            rc["ps_u"][h][:, s, :],
                            es[:, h, s * P:(s + 1) * P],
                            v1_sb[:, kt, 2 * p + h, :],
                            start=(j == 0 and s == s0),
                            stop=(j == last_j and s == NSUB - 1),
                        )
                if rc["last_row"]:
                    # per-sub pipeline: as each q-subtile completes (its
                    # diagonal block), norm + PE-transpose + out-projection
                    # run while the remaining exps stream
                    while (rc["normed"] < NSUB
                           and rc["sub_last"].get(rc["normed"]) == j):
                        s = rc["normed"]
                        norm_emit(rc, s, s + 1, pe_t=True)
                        tt = qt * NSUB + s
                        fillers.extend(
                            (("op", tt, half),
                             make_outproj(tt, half, pe_t=(p, s)))
                            for half in range(2))
                        rc["normed"] += 1
                if j == last_j:
                    if rc["normed"] < NSUB:
                        norm_emit(rc, rc["normed"], NSUB, pe_t=rc["last_row"])
                        if rc["last_row"]:
                            for s in range(rc["normed"], NSUB):
                                tt = qt * NSUB + s
                                fillers.extend(
                                    (("op", tt, half),
                                     make_outproj(tt, half, pe_t=(p, s)))
                                    for half in range(2))
                        rc["normed"] = NSUB
                    done_pairs[qt] += 1
                    if done_pairs[qt] == NPAIR and qt != ROW_ORDER[-1]:
                        fillers.extend(
                            (("op", tt, half), make_outproj(tt, half))
                            for tt in range(qt * NSUB, (qt + 1) * NSUB)
                            for half in range(2))

            # ---- prologue: inline just enough for row (p0, q0) ----
            # kT/qT in two t-halves (one accumulation group each, ranges
            # zero-fill on first touch): the first half only needs the first
            # x t-piece, so q0's first score/exp pieces start ~5us earlier
            pro_k = psA.tile([P, QT], f32, tag="psA", name="pro_k")
            pro_q = psA.tile([P, QT], f32, tag="psA", name="pro_q")
            HQ = QT // 2
            for hh in range(2):
                # all-k then all-q per half: k needs only wk (first DMA) +
                # the x piece, q waits on the later wq load — so the PE
                # starts on the earliest bytes
                for w_sb, box in ((wk_sb, pro_k), (wq_sb, pro_q)):
                    for dt in range(ND):
                        nc.tensor.matmul(
                            box[:, hh * HQ:(hh + 1) * HQ],
                            w_sb[:, dt, 0:P],
                            xt_sb[:, dt, hh * HQ:(hh + 1) * HQ],
                            start=(hh == 0 and dt == 0),
                            stop=(hh == 1 and dt == ND - 1),
                        )
                # split evicts across engines: ACT is idle before the exps
                nc.vector.tensor_copy(
                    kT_sb[0][:, hh * HQ:(hh + 1) * HQ], pro_k[:, hh * HQ:(hh + 1) * HQ])
                nc.scalar.copy(
                    qT_sb[0][:, hh * HQ:(hh + 1) * HQ], pro_q[:, hh * HQ:(hh + 1) * HQ])

            def v_units(lo, hi):
                return [(("v", tt), (lambda tt=tt: emit_v(tt))) for tt in range(lo, hi)]

            # filler queue ordered to match row order q0, q1, q3, q2 so lazy
            # flushes stay small. pair-1's q0 projections lead: they are
            # data-ready with the prologue (wq/wk/xA), while v0..3 wait the
            # later wv DMA — this kills the first-seam exp bubble.
            fillers.extend(proj_units("kT", 1, 0))
            fillers.extend(proj_units("qT", 1, 0))
            fillers.extend(v_units(0, 4))
            fillers.extend(v_units(4, 8))
            for pp in range(NPAIR):
                fillers.extend(proj_units("kT", pp, 1))
                fillers.extend(proj_units("qT", pp, 1))
            fillers.extend(v_units(8, 12))
            fillers.extend(proj_units("kT", 0, 2))
            fillers.extend(proj_units("kT", 0, 3))
            fillers.extend(proj_units("qT", 0, 3))
            fillers.extend(v_units(12, 16))
            fillers.extend(proj_units("kT", 1, 2))
            fillers.extend(proj_units("kT", 1, 3))
            fillers.extend(proj_units("qT", 1, 3))
            fillers.extend(proj_units("qT", 0, 2))
            fillers.extend(proj_units("qT", 1, 2))

            # ---- flat block stream across all rows (q0, q1, q3, q2) ----
            rows = [(p, qt) for qt in ROW_ORDER for p in range(NPAIR)]
            done_pairs = {qt: 0 for qt in ROW_ORDER}
            flat = []
            for p, qt in rows:
                row = vis[qt]
                last_row = (qt == ROW_ORDER[-1] and p == NPAIR - 1)
                if not row:
                    for h in range(2):
                        nc.vector.memset(
                            atT_sb[p][:, qt * NSUB:(qt + 1) * NSUB,
                                      h * Dh:(h + 1) * Dh], 0.0)
                    nc.sync.dma_start_transpose(
                        at_sb[p][:, qt * NSUB * P:(qt + 1) * NSUB * P].rearrange(
                            "p (n c) -> p n c", n=NSUB),
                        atT_sb[p][:, qt * NSUB:(qt + 1) * NSUB, :].rearrange(
                            "p n c -> p (n c)"))
                    done_pairs[qt] += 1
                    if done_pairs[qt] == NPAIR and qt != ROW_ORDER[-1]:
                        fillers.extend(
                            (("op", tt, half), make_outproj(tt, half))
                            for tt in range(qt * NSUB, (qt + 1) * NSUB)
                            for half in range(2))
                    continue
                sub_first, sub_last = _row_subs(row, meta)
                rc = dict(p=p, qt=qt, row=row, sub_first=sub_first,
                          sub_last=sub_last, ps_u=None, es_q={}, normed=0,
                          last_row=last_row)
                flat.extend((rc, j) for j in range(len(row)))

            def _first_pieces(rc, j):
                _kt, pidx = rc["row"][j]
                w0 = 0 if pidx is None else meta[pidx][0]
                return ((w0, HQ), (HQ, QT)) if w0 < HQ else None

            # the first two blocks emit scores/exps in column halves so the
            # exp stream starts as soon as the first qT half is projected
            if flat:
                sc_emit(*flat[0], pieces=_first_pieces(*flat[0]))
            if len(flat) > 1:
                sc_emit(*flat[1], pieces=_first_pieces(*flat[1]))
            for k, (rc, j) in enumerate(flat):
                # scores first so the exp stream never queues behind chunky
                # filler matmuls on the PE
                if k + 2 < len(flat):
                    sc_emit(*flat[k + 2])
                drain(2)
                au_emit(rc, j)
            while fillers:
                drain(len(fillers))
            if debug:
                for p in range(NPAIR):
                    nc.sync.dma_start(dbg["d_qT"][p], qT_sb[p][:])
                    nc.sync.dma_start(dbg["d_kT"][p], kT_sb[p][:])
                    nc.sync.dma_start(dbg["d_atT"][p], atT_sb[p][:])
                    nc.sync.dma_start(dbg["d_at"][p], at_sb[p][:])
                nc.sync.dma_start(dbg["d_v1"][:], v1_sb[:])
    if compile:
        nc.compile()
    return nc


def _host_inputs(x, mask, w_qkv, w_out):
    vis, pm, meta = _block_structure(np.asarray(mask))
    if meta:
        for w0, m_lo, m_hi in meta:
            assert w0 <= m_lo and m_hi <= w0 + P
        pm_win = np.stack([pm[k][:, meta[k][0]:meta[k][0] + P]
                           for k in range(pm.shape[0])])
    else:
        pm_win = pm[:, :, :P]
    pm_c = pm_win.astype(NP_CDT)
    wq_f, wk_f, wv_f = np.split(np.asarray(w_qkv, np.float32), 3, axis=1)
    in_maps = []
    for core in range(N_CORES):
        b = core // 4
        g = core % 4
        cols = slice(g * HPC * Dh, (g + 1) * HPC * Dh)
        in_maps.append({
            "xT": np.ascontiguousarray(np.asarray(x[b], np.float32).T).astype(NP_CDT),
            "wq": wq_f[:, cols].astype(NP_CDT),
            "wk": wk_f[:, cols].astype(NP_CDT),
            "wv": wv_f[:, cols].astype(NP_CDT),
            "wo": np.asarray(w_out, np.float32)[cols, :].astype(NP_CDT),
            "pmask": pm_c,
            "ident": np.eye(P, dtype=NP_CDT),
        })
    return vis, pm, meta, in_maps


def run(x, mask, w_qkv, w_out, trace=False):
    import os
    vis, pm, meta, in_maps = _host_inputs(x, mask, w_qkv, w_out)
    nc = _build_program(vis, pm.shape[0], meta)
    if not trace:
        os.environ["BASS_NEVER_TRACE"] = "1"
    else:
        os.environ.pop("BASS_NEVER_TRACE", None)
    res = run_bass_kernel_spmd(nc, in_maps, core_ids=list(range(N_CORES)), trace=trace)
    parts = [res.results[i]["y"].astype(np.float32) for i in range(N_CORES)]
    out = np.stack([
        parts[0] + parts[1] + parts[2] + parts[3],
        parts[4] + parts[5] + parts[6] + parts[7],
    ]).astype(np.float32)
    return out, res


def kernel(x, mask, w_qkv, w_out):
    out, _ = run(x, mask, w_qkv, w_out, trace=False)
    return out



# revision 81
# speedup vs baseline: 1.0366x; 1.0138x over previous
"""Multi-head causal attention (B=2, T=2048, D=1024, H=16) on 8 trn2 NeuronCores.

Sharding: 8 cores = 2 batches x 4 head-groups (4 heads each). Each core:
  - computes qkv projections for its 4 heads from x[b] (pre-transposed on host),
  - runs masked softmax attention,
  - emits a partial output projection y_part = attn_heads @ w_out[head_rows] (bf16).
Host sums the 4 partial y per batch in fp32.

Schedule highlights (tuned against the TimelineSim cost model):
  - PE matmul cost is moving-column count only; attnU runs in (q-partition
    x 65) orientation: stationary = exp-scores sub-tile [128k x 128q],
    moving = [v(64) | ones(1)] so softmax denominators accumulate as psum
    column 64 at full array utilization (no wasted columns).
  - PSUM accumulation groups are per 2KB zero-region (whole bank): one
    start/stop group per head-bank per row; sub-ranges zero-fill on first
    touch.
  - One ACT exp per (pair, qtile, ktile) covering both heads ([128, 2, 512]
    psum tile spanning 2 banks) to halve ACT fixed overhead.
  - Normalization: psum col 64 -> custom-DVE reciprocal -> stride-0
    broadcast multiply into atT (q-part, dims); atT -> at via batched
    128x128-block DMA-transpose for the out-proj stationary. No
    cross-partition shuffles.
  - All (pair, q-row) block streams are flattened into one globally
    software-pipelined sequence (scores lookahead 2, bounded by 2 psS
    bufs = 4 banks) so the exp stream crosses row seams without stalling;
    the final row norms/transposes/out-projects per q-subtile as each
    subtile's accumulation completes at its diagonal block.
  - qkv/out projections and v-tiles are filler units drained into the
    exp-paced attention loop (rationed when scarce so the exp-heavy late
    rows stay covered); rows run in order q0, q1, q3, q2.
  - Input DMAs on one queue in strict need-order (wk/wq pair-0 halves,
    first x quarter split by t-halves, pair-1 weight halves, wv, compact
    pmask window, rest); the prologue runs all-k-then-all-q per half so
    the PE starts on the earliest bytes. pmask carries only the 128-wide
    partial window of each pattern (4x less startup DMA). y stores in
    bf16 (summed in fp32 on host); mask multiplies offloaded to GpSimd.
"""
import sys
sys.path.insert(0, "/opt/trn_rl_repo")

import numpy as np
import ml_dtypes

import concourse.bass as bass
import concourse.mybir as mybir
import concourse.tile as tile
from concourse import bacc
from concourse.bass_utils import run_bass_kernel_spmd

B, T, D, H, Dh = 2, 2048, 1024, 16, 64
P = 128
QT = 512              # q-tile width (score tile free dim)
NQ = T // QT          # 4
NKT = T // P          # 16
ND = D // P           # 8
HPC = 4               # heads per core
NPAIR = HPC // 2      # head pairs per core
NSUB = QT // P        # 4 q-subtiles per q-tile
VW = Dh + 1           # attnU moving width: 64 v dims + 1 ones col
N_CORES = 8

f32 = mybir.dt.float32
bf16 = mybir.dt.bfloat16
CDT = bf16
NP_CDT = ml_dtypes.bfloat16


def _block_structure(mask: np.ndarray):
    """Classify maskT (k,q) blocks: per q-tile a list of (kt, pattern_idx|None).

    For each unique partial pattern derive (w0, m_lo, m_hi): w0 leading
    all-masked columns (exp skipped), [m_lo, m_hi) the column range needing
    the mask multiply.
    """
    maskT = (mask != 0).T.astype(np.float32)  # [k, q] visibility
    vis = []
    patterns = []
    meta = []
    pat_index = {}
    for qt in range(NQ):
        row = []
        for kt in range(NKT):
            blk = maskT[kt * P:(kt + 1) * P, qt * QT:(qt + 1) * QT]
            s = blk.sum()
            if s == 0:
                continue
            if s == blk.size:
                row.append((kt, None))
            else:
                key = blk.tobytes()
                if key not in pat_index:
                    pat_index[key] = len(patterns)
                    patterns.append(blk)
                    col_any = blk.any(axis=0)
                    col_all = blk.all(axis=0)
                    w0 = int(np.argmax(col_any)) if col_any.any() else QT
                    partial_cols = np.nonzero(col_any & ~col_all)[0]
                    if partial_cols.size:
                        m_lo, m_hi = int(partial_cols[0]), int(partial_cols[-1]) + 1
                    else:
                        m_lo = m_hi = 0
                    meta.append((w0, m_lo, m_hi))
                row.append((kt, pat_index[key]))
        vis.append(row)
    if patterns:
        pm = np.stack(patterns)
    else:
        pm = np.zeros((1, P, QT), np.float32)
    return vis, pm, meta


def _row_subs(row, meta):
    """Per q-subtile s: (first_j, last_j) of blocks covering s, or None."""
    sub_first = {}
    sub_last = {}
    for j, (kt, pidx) in enumerate(row):
        w0 = 0 if pidx is None else meta[pidx][0]
        for s in range(w0 // P, NSUB):
            if s not in sub_first:
                sub_first[s] = j
            sub_last[s] = j
    return sub_first, sub_last


def _build_program(vis, n_pm, meta=(), compile=True, debug=False):
    nc = bacc.Bacc() if compile else bass.Bass()
    xT = nc.declare_dram_parameter("xT", [D, T], CDT, isOutput=False)
    wq = nc.declare_dram_parameter("wq", [D, HPC * Dh], CDT, isOutput=False)
    wk = nc.declare_dram_parameter("wk", [D, HPC * Dh], CDT, isOutput=False)
    wv = nc.declare_dram_parameter("wv", [D, HPC * Dh], CDT, isOutput=False)
    wo = nc.declare_dram_parameter("wo", [HPC * Dh, D], CDT, isOutput=False)
    # pmask holds only the P-wide partial window of each pattern (cols
    # [w0, w0+P) of the full tile) — the rest is all-ones/all-zeros
    pmask = nc.declare_dram_parameter("pmask", [n_pm, P, P], CDT, isOutput=False)
    ident = nc.declare_dram_parameter("ident", [P, P], CDT, isOutput=False)
    y = nc.declare_dram_parameter("y", [T, D], CDT, isOutput=True)
    if debug:
        dbg = {
            "d_qT": nc.declare_dram_parameter("d_qT", [NPAIR, P, T], CDT, isOutput=True),
            "d_kT": nc.declare_dram_parameter("d_kT", [NPAIR, P, T], CDT, isOutput=True),
            "d_v1": nc.declare_dram_parameter("d_v1", [P, NKT, HPC, VW], CDT, isOutput=True),
            "d_atT": nc.declare_dram_parameter("d_atT", [NPAIR, P, NKT, P], CDT, isOutput=True),
            "d_at": nc.declare_dram_parameter("d_at", [NPAIR, P, T], CDT, isOutput=True),
        }

    inv_sqrt_dh = 1.0 / float(np.sqrt(Dh))
    ROW_ORDER = (0, 1, 3, 2)

    with tile.TileContext(nc) as tc:
        with (
            tc.tile_pool(name="persist", bufs=1) as persist,
            tc.tile_pool(name="work", bufs=3) as work,
            tc.tile_pool(name="psA", bufs=2, space="PSUM") as psA,
            tc.tile_pool(name="psS", bufs=2, space="PSUM") as psS,
            tc.tile_pool(name="psU", bufs=1, space="PSUM") as psU,
        ):
            # ---- persistent SBUF tensors ----
            xt_sb = persist.tile([P, ND, T], CDT, tag="xt")
            wq_sb = persist.tile([P, ND, HPC * Dh], CDT, tag="wq")
            wk_sb = persist.tile([P, ND, HPC * Dh], CDT, tag="wk")
            wv_sb = persist.tile([P, ND, HPC * Dh], CDT, tag="wv")
            wo_sb = persist.tile([P, NPAIR, D], CDT, tag="wo")
            pm_sb = persist.tile([P, n_pm, P], CDT, tag="pm")
            qT_sb = [persist.tile([P, T], CDT, tag=f"qT{p}", name=f"qT{p}") for p in range(NPAIR)]
            kT_sb = [persist.tile([P, T], CDT, tag=f"kT{p}", name=f"kT{p}") for p in range(NPAIR)]
            # atT: (q-part, per t-tile: 2 heads x 64 dims); at: transposed
            atT_sb = [persist.tile([P, NKT, P], CDT, tag=f"aT{p}", name=f"aT{p}") for p in range(NPAIR)]
            at_sb = [persist.tile([P, T], CDT, tag=f"at{p}", name=f"at{p}") for p in range(NPAIR)]
            # v1: per (k-tile, head): [128 k, 65] = v dims 0:64, ones col 64.
            v1_sb = persist.tile([P, NKT, HPC, VW], CDT, tag="v1")
            id_sb = persist.tile([P, P], CDT, tag="id")

            # ones columns (tiny strided memset; no DMA dependency)
            nc.vector.memset(v1_sb[:, :, :, Dh:VW], 1.0)

            # dummy exp to pre-trigger the ACT exp-table load off the
            # critical path (reads the just-memset ones column)
            scratch = persist.tile([P, 1], CDT, tag="scr")
            nc.scalar.activation(
                scratch[:], v1_sb[:, 0, 0, Dh:VW],
                mybir.ActivationFunctionType.Exp)

            xr = xT.rearrange("(o p) t -> p o t", p=P)
            wqr = wq.rearrange("(o p) e -> p o e", p=P)
            wkr = wk.rearrange("(o p) e -> p o e", p=P)
            # input DMAs on the sync queue, strictly need-ordered. The qk
            # weights are split by head-pair so the prologue (pair 0 only)
            # unblocks after 0.5MB of weights instead of 1MB; pmask is the
            # compact partial window (4x smaller).
            # xA1 ahead of wq0: the prologue's k half needs only wk0 + xA1,
            # and wq0 lands during the k matmuls + evict, just in time for
            # the q half
            nc.sync.dma_start(wk_sb[:, :, 0:P], wkr[:, :, 0:P])
            nc.sync.dma_start(xt_sb[:, :, 0:QT // 2], xr[:, :, 0:QT // 2])
            nc.sync.dma_start(wq_sb[:, :, 0:P], wqr[:, :, 0:P])
            nc.sync.dma_start(xt_sb[:, :, QT // 2:QT], xr[:, :, QT // 2:QT])
            nc.sync.dma_start(wk_sb[:, :, P:2 * P], wkr[:, :, P:2 * P])
            nc.sync.dma_start(wq_sb[:, :, P:2 * P], wqr[:, :, P:2 * P])
            nc.sync.dma_start(wv_sb[:], wv.rearrange("(o p) e -> p o e", p=P))
            nc.sync.dma_start(pm_sb[:], pmask.rearrange("n p q -> p n q"))
            for c in range(1, NQ):
                nc.sync.dma_start(xt_sb[:, :, c * QT:(c + 1) * QT],
                                  xr[:, :, c * QT:(c + 1) * QT])
            nc.sync.dma_start(wo_sb[:], wo.rearrange("(o p) e -> p o e", p=P))
            nc.sync.dma_start(id_sb[:], ident[:])


            # ---- filler queue: PE-side work interleaved into ACT-paced ----
            # ---- attention steps                                        ----
            fillers = []  # list of (key, thunk); emitted in order

            def drain(k):
                # when fillers run low, ration them: the exp-heavy late rows
                # need leftover PE work to cover their ACT-paced stretches.
                # when the queue is deep, pre-drain harder so seam flushes
                # stay small.
                if len(fillers) > 60:
                    k += 1
                elif k > 1 and len(fillers) < 30:
                    k = 1
                for _ in range(min(k, len(fillers))):
                    fillers.pop(0)[1]()

            def flush_through(pred):
                while any(pred(key) for key, _ in fillers):
                    fillers.pop(0)[1]()

            # ---- v = x @ wv for one t-tile ----
            def emit_v(tt):
                ps_v = psA.tile([P, QT], f32, tag="psA", name=f"psv{tt}")
                for dt in range(ND):
                    nc.tensor.matmul(
                        ps_v[:, :HPC * Dh],
                        xt_sb[:, dt, tt * P:(tt + 1) * P],
                        wv_sb[:, dt, :],
                        start=(dt == 0),
                        stop=(dt == ND - 1),
                    )
                ps_vh = ps_v[:, :HPC * Dh].rearrange("p (h e) -> p h e", h=HPC)
                # alternate DVE/ACT evictions: all-DVE makes the PE's filler
                # stream stall on psA turnover behind the Vector queue
                if tt % 2:
                    nc.scalar.copy(v1_sb[:, tt, :, 0:Dh], ps_vh[:])
                else:
                    nc.vector.tensor_copy(v1_sb[:, tt, :, 0:Dh], ps_vh[:])

            # ---- kT or qT projection for (pair, nt) as 9 filler units ----
            def proj_units(kind, p, nt):
                w_sb = wk_sb if kind == "kT" else wq_sb
                out_sb = kT_sb[p] if kind == "kT" else qT_sb[p]
                ps_box = []

                def mm(dt):
                    if dt == 0:
                        ps_box.append(psA.tile(
                            [P, QT], f32, tag="psA", name=f"ps{kind}{p}_{nt}"))
                    nc.tensor.matmul(
                        ps_box[0],
                        w_sb[:, dt, p * P:(p + 1) * P],
                        xt_sb[:, dt, nt * QT:(nt + 1) * QT],
                        start=(dt == 0),
                        stop=(dt == ND - 1),
                    )

                def evict():
                    if (nt + (0 if kind == "kT" else 1)) % 2:
                        nc.scalar.copy(
                            out_sb[:, nt * QT:(nt + 1) * QT], ps_box[0])
                    else:
                        nc.vector.tensor_copy(
                            out_sb[:, nt * QT:(nt + 1) * QT], ps_box[0])

                key = (kind, p, nt)

                def mk(dt):
                    return lambda: mm(dt)

                units = [(key, mk(dt)) for dt in range(ND)]
                units.append((key, evict))
                return units

            # ---- out-projection for one t-tile/half (as filler) ----
            # both halves share one ysb tile; a single [P, 1024] DMA per
            # t-tile (2KB rows) halves the sync-queue config count
            y_tiles = {}

            def make_outproj(tt, half, pe_t=None):
                def go():
                    if pe_t is not None and half == 0:
                        # last row: transpose this tile's atT on the PE —
                        # drained >= one block after the norm wrote atT, so
                        # the wait is already satisfied and the DMA
                        # round-trip + completion-sem latency is skipped
                        pp, s = pe_t
                        psT = psA.tile([P, P], CDT, tag="psA", name=f"peT{s}")
                        nc.tensor.transpose(
                            psT, atT_sb[pp][:, tt, :], id_sb[:])
                        nc.vector.tensor_copy(
                            at_sb[pp][:, tt * P:(tt + 1) * P], psT)
                    ps_y = psA.tile([P, QT], f32, tag="psA", name=f"psy{tt}_{half}")
                    for p in range(NPAIR):
                        nc.tensor.matmul(
                            ps_y[:],
                            at_sb[p][:, tt * P:(tt + 1) * P],
                            wo_sb[:, p, half * QT:(half + 1) * QT],
                            start=(p == 0),
                            stop=(p == NPAIR - 1),
                        )
                    if pe_t is not None:
                        # last row: store each half as soon as its cast lands
                        # so the final cast and y DMA pipeline
                        yh = work.tile([P, QT], CDT, tag="y", name=f"y{tt}_{half}")
                        nc.vector.tensor_copy(yh[:], ps_y[:])
                        nc.sync.dma_start(
                            y[tt * P:(tt + 1) * P, half * QT:(half + 1) * QT],
                            yh[:])
                        return
                    if tt not in y_tiles:
                        y_tiles[tt] = [
                            work.tile([P, 2, QT], CDT, tag="y", name=f"y{tt}"), 0]
                    ent = y_tiles[tt]
                    nc.vector.tensor_copy(ent[0][:, half], ps_y[:])
                    ent[1] += 1
                    if ent[1] == 2:
                        nc.sync.dma_start(
                            y[tt * P:(tt + 1) * P, :],
                            ent[0].rearrange("p a b -> p (a b)"))
                return go

            # ---- globally pipelined attention: one flat block stream ----
            def norm_emit(rc, lo, hi, pe_t=False):
                p, qt, sub_first, ps_u = rc["p"], rc["qt"], rc["sub_first"], rc["ps_u"]
                n = hi - lo
                den = work.tile([P, 2, n, 1], f32, tag="den", name=f"den{p}_{qt}_{lo}")
                rep = work.tile([P, 2, n, 1], f32, tag="rep", name=f"rep{p}_{qt}_{lo}")
                for h in range(2):
                    nc.vector.tensor_copy(den[:, h], ps_u[h][:, lo:hi, Dh:VW])
                    for s in range(lo, hi):
                        if s not in sub_first:
                            nc.vector.memset(den[:, h, s - lo], 1.0)
                nc.vector.reciprocal_approx_fast(
                    rep.rearrange("p a b c -> p (a b c)"),
                    den.rearrange("p a b c -> p (a b c)"))
                for h in range(2):
                    nc.vector.tensor_mul(
                        atT_sb[p][:, qt * NSUB + lo:qt * NSUB + hi, h * Dh:(h + 1) * Dh],
                        ps_u[h][:, lo:hi, 0:Dh],
                        rep[:, h].broadcast_to((P, n, Dh)),
                    )
                    for s in range(lo, hi):
                        if s not in sub_first:
                            nc.vector.memset(
                                atT_sb[p][:, qt * NSUB + s, h * Dh:(h + 1) * Dh], 0.0)
                if pe_t:
                    # last row: the transpose happens on the PE inside the
                    # out-projection filler — only the normalization here
                    return
                # batched transpose overlaps remaining work
                nc.sync.dma_start_transpose(
                    at_sb[p][:, (qt * NSUB + lo) * P:(qt * NSUB + hi) * P].rearrange(
                        "p (n c) -> p n c", n=n),
                    atT_sb[p][:, qt * NSUB + lo:qt * NSUB + hi, :].rearrange(
                        "p n c -> p (n c)"))

            def sc_emit(rc, j, pieces=None):
                p, qt, row = rc["p"], rc["qt"], rc["row"]
                kt, pidx = row[j]
                if j == 0:
                    flush_through(lambda key, p=p, qt=qt: (
                        key[0] == "qT" and key[1] == p and key[2] == qt))
                flush_through(lambda key, p=p, kt=kt: (
                    key[0] == "kT" and key[1] == p and key[2] <= kt // NSUB))
                w0 = 0 if pidx is None else meta[pidx][0]
                ps_s = psS.tile([P, 2, QT], f32, tag="s", name=f"s_{p}_{qt}_{kt}")
                es = work.tile([P, 2, QT], CDT, tag="es", name=f"es_{p}_{qt}_{kt}")
                if w0 % P:
                    # stale data in the leading partial subtile
                    nc.vector.memset(es[:, :, (w0 // P) * P:w0], 0.0)
                # pieces: column ranges emitted as independent score/exp
                # passes (used by the first blocks so the exp stream starts
                # before the full-width qT projection lands)
                for lo, hi in (pieces or ((w0, QT),)):
                    for h in range(2):
                        base = h * Dh
                        nc.tensor.matmul(
                            ps_s[:, h, lo:hi],
                            kT_sb[p][base:base + Dh, kt * P:(kt + 1) * P],
                            qT_sb[p][base:base + Dh, qt * QT + lo:qt * QT + hi],
                            start=True,
                            stop=True,
                            tile_position=(base, 0),
                        )
                    nc.scalar.activation(
                        es[:, :, lo:hi], ps_s[:, :, lo:hi],
                        mybir.ActivationFunctionType.Exp,
                        scale=inv_sqrt_dh,
                    )
                    if pidx is not None:
                        _w0, m_lo, m_hi = meta[pidx]
                        ml, mh = max(m_lo, lo), min(m_hi, hi)
                        if mh > ml:
                            pmb = pm_sb[:, pidx:pidx + 1,
                                        ml - _w0:mh - _w0].broadcast_to(
                                (P, 2, mh - ml))
                            if rc["last_row"] and j == len(rc["row"]) - 1:
                                # kernel tail: DVE is idle here and faster
                                # per element than GpSimd — shortens the
                                # final block's exp->mask->attnU chain
                                nc.vector.tensor_mul(
                                    es[:, :, ml:mh], es[:, :, ml:mh], pmb)
                            else:
                                nc.gpsimd.tensor_mul(
                                    es[:, :, ml:mh], es[:, :, ml:mh], pmb)
                rc["es_q"][j] = (es, w0)

            # PSUM accumulation groups are per 2KB zero-region (a whole
            # bank): start marks the bank pending-zero, each later matmul
            # zero-fills its range on first touch and accumulates after.
            # So: one group per head-bank per row.
            def au_emit(rc, j):
                p, qt, row = rc["p"], rc["qt"], rc["row"]
                kt, pidx = row[j]
                flush_through(lambda key, kt=kt: (
                    key[0] == "v" and key[1] <= kt))
                if rc["ps_u"] is None:
                    rc["ps_u"] = [
                        psU.tile([P, NSUB, VW], f32, tag=f"u{h}", name=f"u{h}_{p}_{qt}")
                        for h in range(2)
                    ]
                es, w0 = rc["es_q"].pop(j)
                s0 = w0 // P
                last_j = len(row) - 1
                for h in range(2):
                    for s in range(s0, NSUB):
                        nc.tensor.matmul(
                            rc["ps_u"][h][:, s, :],
                            es[:, h, s * P:(s + 1) * P],
                            v1_sb[:, kt, 2 * p + h, :],
                            start=(j == 0 and s == s0),
                            stop=(j == last_j and s == NSUB - 1),
                        )
                if rc["last_row"]:
                    # per-sub pipeline: as each q-subtile completes (its
                    # diagonal block), norm + PE-transpose + out-projection
                    # run while the remaining exps stream
                    while (rc["normed"] < NSUB
                           and rc["sub_last"].get(rc["normed"]) == j):
                        s = rc["normed"]
                        norm_emit(rc, s, s + 1, pe_t=True)
                        tt = qt * NSUB + s
                        fillers.extend(
                            (("op", tt, half),
                             make_outproj(tt, half, pe_t=(p, s)))
                            for half in range(2))
                        rc["normed"] += 1
                if j == last_j:
                    if rc["normed"] < NSUB:
                        norm_emit(rc, rc["normed"], NSUB, pe_t=rc["last_row"])
                        if rc["last_row"]:
                            for s in range(rc["normed"], NSUB):
                                tt = qt * NSUB + s
                                fillers.extend(
                                    (("op", tt, half),
                                     make_outproj(tt, half, pe_t=(p, s)))
                                    for half in range(2))
                        rc["normed"] = NSUB
                    done_pairs[qt] += 1
                    if done_pairs[qt] == NPAIR and qt != ROW_ORDER[-1]:
                        fillers.extend(
                            (("op", tt, half), make_outproj(tt, half))
                            for tt in range(qt * NSUB, (qt + 1) * NSUB)
                            for half in range(2))

            # ---- prologue: inline just enough for row (p0, q0) ----
            # kT/qT in two t-halves (one accumulation group each, ranges
            # zero-fill on first touch): the first half only needs the first
            # x t-piece, so q0's first score/exp pieces start ~5us earlier
            pro_k = psA.tile([P, QT], f32, tag="psA", name="pro_k")
            pro_q = psA.tile([P, QT], f32, tag="psA", name="pro_q")
            HQ = QT // 2
            for hh in range(2):
                # all-k then all-q per half: k needs only wk (first DMA) +
                # the x piece, q waits on the later wq load — so the PE
                # starts on the earliest bytes
                for w_sb, box in ((wk_sb, pro_k), (wq_sb, pro_q)):
                    for dt in range(ND):
                        nc.tensor.matmul(
                            box[:, hh * HQ:(hh + 1) * HQ],
                            w_sb[:, dt, 0:P],
                            xt_sb[:, dt, hh * HQ:(hh + 1) * HQ],
                            start=(hh == 0 and dt == 0),
                            stop=(hh == 1 and dt == ND - 1),
                        )
                # split evicts across engines: ACT is idle before the exps
                nc.vector.tensor_copy(
                    kT_sb[0][:, hh * HQ:(hh + 1) * HQ], pro_k[:, hh * HQ:(hh + 1) * HQ])
                nc.scalar.copy(
                    qT_sb[0][:, hh * HQ:(hh + 1) * HQ], pro_q[:, hh * HQ:(hh + 1) * HQ])

            def v_units(lo, hi):
                return [(("v", tt), (lambda tt=tt: emit_v(tt))) for tt in range(lo, hi)]

            # filler queue ordered to match row order q0, q1, q3, q2 so lazy
            # flushes stay small. pair-1's q0 projections lead: they are
            # data-ready with the prologue (wq/wk/xA), while v0..3 wait the
            # later wv DMA — this kills the first-seam exp bubble.
            fillers.extend(proj_units("kT", 1, 0))
            fillers.extend(proj_units("qT", 1, 0))
            fillers.extend(v_units(0, 4))
            fillers.extend(v_units(4, 8))
            for pp in range(NPAIR):
                fillers.extend(proj_units("kT", pp, 1))
                fillers.extend(proj_units("qT", pp, 1))
            fillers.extend(v_units(8, 12))
            fillers.extend(proj_units("kT", 0, 2))
            fillers.extend(proj_units("kT", 0, 3))
            fillers.extend(proj_units("qT", 0, 3))
            fillers.extend(v_units(12, 16))
            fillers.extend(proj_units("kT", 1, 2))
            fillers.extend(proj_units("kT", 1, 3))
            fillers.extend(proj_units("qT", 1, 3))
            fillers.extend(proj_units("qT", 0, 2))
            fillers.extend(proj_units("qT", 1, 2))

            # ---- flat block stream across all rows (q0, q1, q3, q2) ----
            rows = [(p, qt) for qt in ROW_ORDER for p in range(NPAIR)]
            done_pairs = {qt: 0 for qt in ROW_ORDER}
            flat = []
            for p, qt in rows:
                row = vis[qt]
                last_row = (qt == ROW_ORDER[-1] and p == NPAIR - 1)
                if not row:
                    for h in range(2):
                        nc.vector.memset(
                            atT_sb[p][:, qt * NSUB:(qt + 1) * NSUB,
                                      h * Dh:(h + 1) * Dh], 0.0)
                    nc.sync.dma_start_transpose(
                        at_sb[p][:, qt * NSUB * P:(qt + 1) * NSUB * P].rearrange(
                            "p (n c) -> p n c", n=NSUB),
                        atT_sb[p][:, qt * NSUB:(qt + 1) * NSUB, :].rearrange(
                            "p n c -> p (n c)"))
                    done_pairs[qt] += 1
                    if done_pairs[qt] == NPAIR and qt != ROW_ORDER[-1]:
                        fillers.extend(
                            (("op", tt, half), make_outproj(tt, half))
                            for tt in range(qt * NSUB, (qt + 1) * NSUB)
                            for half in range(2))
                    continue
                sub_first, sub_last = _row_subs(row, meta)
                rc = dict(p=p, qt=qt, row=row, sub_first=sub_first,
                          sub_last=sub_last, ps_u=None, es_q={}, normed=0,
                          last_row=last_row)
                flat.extend((rc, j) for j in range(len(row)))

            def _first_pieces(rc, j):
                _kt, pidx = rc["row"][j]
                w0 = 0 if pidx is None else meta[pidx][0]
                return ((w0, HQ), (HQ, QT)) if w0 < HQ else None

            # the first two blocks emit scores/exps in column halves so the
            # exp stream starts as soon as the first qT half is projected
            if flat:
                sc_emit(*flat[0], pieces=_first_pieces(*flat[0]))
            if len(flat) > 1:
                sc_emit(*flat[1], pieces=_first_pieces(*flat[1]))
            for k, (rc, j) in enumerate(flat):
                # scores first so the exp stream never queues behind chunky
                # filler matmuls on the PE
                if k + 2 < len(flat):
                    sc_emit(*flat[k + 2])
                drain(2)
                au_emit(rc, j)
            while fillers:
                drain(len(fillers))
            if debug:
                for p in range(NPAIR):
                    nc.sync.dma_start(dbg["d_qT"][p], qT_sb[p][:])
                    nc.sync.dma_start(dbg["d_kT"][p], kT_sb[p][:])
                    nc.sync.dma_start(dbg["d_atT"][p], atT_sb[p][:])
                    nc.sync.dma_start(dbg["d_at"][p], at_sb[p][:])
                nc.sync.dma_start(dbg["d_v1"][:], v1_sb[:])
    if compile:
        nc.compile()
    return nc


def _host_inputs(x, mask, w_qkv, w_out):
    vis, pm, meta = _block_structure(np.asarray(mask))
    if meta:
        for w0, m_lo, m_hi in meta:
            assert w0 <= m_lo and m_hi <= w0 + P
        pm_win = np.stack([pm[k][:, meta[k][0]:meta[k][0] + P]
                           for k in range(pm.shape[0])])
    else:
        pm_win = pm[:, :, :P]
    pm_c = pm_win.astype(NP_CDT)
    wq_f, wk_f, wv_f = np.split(np.asarray(w_qkv, np.float32), 3, axis=1)
    in_maps = []
    for core in range(N_CORES):
        b = core // 4
        g = core % 4
        cols = slice(g * HPC * Dh, (g + 1) * HPC * Dh)
        in_maps.append({
            "xT": np.ascontiguousarray(np.asarray(x[b], np.float32).T).astype(NP_CDT),
            "wq": wq_f[:, cols].astype(NP_CDT),
            "wk": wk_f[:, cols].astype(NP_CDT),
            "wv": wv_f[:, cols].astype(NP_CDT),
            "wo": np.asarray(w_out, np.float32)[cols, :].astype(NP_CDT),
            "pmask": pm_c,
            "ident": np.eye(P, dtype=NP_CDT),
        })
    return vis, pm, meta, in_maps


def run(x, mask, w_qkv, w_out, trace=False):
    import os
    vis, pm, meta, in_maps = _host_inputs(x, mask, w_qkv, w_out)
    nc = _build_program(vis, pm.shape[0], meta)
    if not trace:
        os.environ["BASS_NEVER_TRACE"] = "1"
    else:
        os.environ.pop("BASS_NEVER_TRACE", None)
    res = run_bass_kernel_spmd(nc, in_maps, core_ids=list(range(N_CORES)), trace=trace)
    parts = [res.results[i]["y"].astype(np.float32) for i in range(N_CORES)]
    out = np.stack([
        parts[0] + parts[1] + parts[2] + parts[3],
        parts[4] + parts[5] + parts[6] + parts[7],
    ]).astype(np.float32)
    return out, res


def kernel(x, mask, w_qkv, w_out):
    out, _ = run(x, mask, w_qkv, w_out, trace=False)
    return out

